# revision 31
# baseline (speedup 1.0000x reference)
"""AtomicConvolution Trainium2 kernel (8 NeuronCores, SPMD, no collectives).

Sharding: N-shard. Core r handles atoms [256r, 256r+256) for ALL 16 batches.
The X coordinate table (tiny) is replicated per core, so the neighbor gather
is core-local and the batch-norm moments over axis 0 (batch) are core-local
too (each core holds all 16 batches for its atoms). No cross-core traffic.

Per-core pipeline (the GPSIMD ap_gather at ~23ns/index is the bottleneck;
everything else is scheduled to hide under it):
  - gather table tbl[16g+c, beta*2048 + j] = plane c of X[2g+beta, j]
    (partition-group g owns batches {2g, 2g+1}; c=0,1,2 -> x,y,z; c=3 -> 0)
  - ap_gather (d=1, 8 chunks of 4096 idx/group, 4-deep output ring) pulls
    all 16 channels per index -> x,y,z planes gathered in one read
  - dx = gathered - centers (broadcast AP over m), squared in place;
    R^2 = PE ones-block matmul reducing the 4 channels; ACT Sqrt;
    DRAM round-trip compacts R to [128, 2048] with partition p = 8b + nb,
    free = (ns, m), atom n = 256r + 32 nb + ns
  - rsf_l = exp(-re(R-rs)^2) * 0.5*(cos(pi R/rc)+1) * [R<=rc]:
      u = R^2 - 2 rs R  (DVE stt); K' = Exp(-re*u - re*rs^2 + ln 0.5)  (ACT)
      cos(u') via Sin(pi/2 - Relu(pi - pi R/rc)): exact cutoff, clamped arg
      rsf = (cos - 1) * K' = -K'*FC  (negation absorbed in the BN subtract)
  - per l: ONE broadcast multiply against the packed 4-type mask + ONE
    segmented reduce into a transposed sym accumulator (f32 throughout)
  - BN over the 16 batches: PE stride-8 partition reductions + broadcasts;
    the final multiply writes through a strided AP to restore (ns, a*12+l)
    output order. Stages run per 2-gather-chunk window (last one split in
    half) so only a 256-column stage trails the final gather.
"""
import sys

if '/opt/trn_rl_repo' not in sys.path:
    sys.path.insert(0, '/opt/trn_rl_repo')

import math
import numpy as np

import concourse.bacc as bacc
import concourse.bass as bass
import concourse.mybir as mybir
from concourse import library_config
from concourse.tile import TileContext

F32 = mybir.dt.float32
BF16 = mybir.dt.bfloat16
I16 = mybir.dt.int16
AF = mybir.ActivationFunctionType
ALU = mybir.AluOpType

P = 128
B, N, M, L, A = 16, 2048, 64, 12, 4
NSH = N // 8                 # atoms per core = 256
NCHUNK = 8                   # gather chunks per core
CI = 4096                    # indices per group per chunk
TF = 2048                    # compacted R free size (= 32 ns * 64 m)
NFEAT = A * L                # 48
OUTF = 32 * NFEAT            # 1536 output cols per partition
ATOM_TYPES = (1, 6, 7, 8)
BN_EPS = 1e-3
PI = math.pi
GCH_BUFS = 6


def build_nc(rc_v, rs_v, re_v, reps=None, ablate=()):
    """Build the per-core graph. rc/rs/re are baked in as immediates.
    reps: if set, wrap the whole body in a HW For_i loop (for benchmarking).
    ablate: subset of {"gather","prod","mm","quarter"} to skip (profiling)."""
    ablate = set(ablate)
    rc_v = [float(x) for x in rc_v]
    rs_v = [float(x) for x in rs_v]
    re_v = [float(x) for x in re_v]
    rc_groups = {}
    for l, v in enumerate(rc_v):
        rc_groups.setdefault(v, []).append(l)
    rc_list = list(rc_groups.keys())
    rcg_of_l = {}
    for gi, v in enumerate(rc_list):
        for l in rc_groups[v]:
            rcg_of_l[l] = gi

    nc = bacc.Bacc()
    tbl_in = nc.declare_dram_parameter("tbl", [P, 2 * N], F32, isOutput=False)
    gidx_in = nc.declare_dram_parameter("gidx", [P, TF], I16, isOutput=False)
    cen_in = nc.declare_dram_parameter("cen", [P, 2 * NSH], F32, isOutput=False)
    zc_in = nc.declare_dram_parameter("zc", [P, TF], F32, isOutput=False)
    wq_in = nc.declare_dram_parameter("wq", [P, 8], F32, isOutput=False)
    bnred_in = nc.declare_dram_parameter("bnred", [P, 8], F32, isOutput=False)
    bnbc_in = nc.declare_dram_parameter("bnbc", [8, P], F32, isOutput=False)
    cb_in = nc.declare_dram_parameter("cbias", [P, 16], F32, isOutput=False)
    out_ext = nc.declare_dram_parameter("out", [P, OUTF], F32, isOutput=True)

    rr_dram = nc.dram_tensor("rr", [NCHUNK, 8, CI], F32)

    import contextlib
    with TileContext(nc) as tc:
        with tc.tile_pool(name="sbuf", bufs=1) as pool, \
             tc.tile_pool(name="psum", bufs=1, space="PSUM") as psum:
            nc.gpsimd.load_library(library_config.ap_gather)
            loop_cm = tc.For_i(0, reps, 1) if reps else contextlib.nullcontext()
            _body_build(nc, tc, pool, psum, loop_cm,
                        tbl_in, gidx_in, cen_in, zc_in, wq_in,
                        bnred_in, bnbc_in, cb_in, out_ext, rr_dram,
                        rc_list, rcg_of_l, rs_v, re_v, ablate)
    nc.compile()
    return nc


def _body_build(nc, tc, pool, psum, loop_cm,
                tbl_in, gidx_in, cen_in, zc_in, wq_in,
                bnred_in, bnbc_in, cb_in, out_ext, rr_dram,
                rc_list, rcg_of_l, rs_v, re_v, ablate=()):
    with loop_cm:
            tbl = pool.tile([P, 2 * N], F32)
            gidx = pool.tile([P, TF], I16)
            cen = pool.tile([P, 2 * NSH], F32)
            zc = pool.tile([P, TF], F32)
            wq = pool.tile([P, 8], F32)
            bnred = pool.tile([P, 8], F32)
            bnbc = pool.tile([8, P], F32)
            cb = pool.tile([P, 16], F32)
            for t, src in [(tbl, tbl_in), (gidx, gidx_in), (cen, cen_in),
                           (zc, zc_in), (wq, wq_in),
                           (bnred, bnred_in), (bnbc, bnbc_in), (cb, cb_in)]:
                nc.sync.dma_start(out=t[:], in_=src[:])

            sym = pool.tile([P, OUTF], F32)
            Rt = pool.tile([P, TF], F32)
            cen_pitch = cen[:].ap[0][0]

            for k in range(NCHUNK):
                # ---- gather chunk k: 4096 idx per group
                gch = pool.tile([P, CI], F32, tag="gch", bufs=GCH_BUFS)
                if "gather" not in ablate:
                    nc.gpsimd.ap_gather(
                        out_ap=gch[:], in_ap=tbl[:],
                        idxs_ap=gidx[:, 256 * k:256 * (k + 1)],
                        channels=P, num_elems=2 * N, d=1, num_idxs=CI)
                else:
                    nc.vector.memset(gch[:], 1.0)

                # ---- in-place: gch = (gch - centers)^2  (ch3 is 0-0=0)
                for beta in range(2):
                    if "prod" in ablate:
                        break
                    cen_ap = bass.AP(
                        cen.tensor,
                        cen[:].offset + 256 * beta + 4 * k,
                        [[cen_pitch, P], [32, 8], [1, 4], [0, 64]])
                    nc.vector.tensor_tensor(
                        out=gch[:, 2048 * beta:2048 * (beta + 1)],
                        in0=gch[:, 2048 * beta:2048 * (beta + 1)],
                        in1=cen_ap, op=ALU.subtract)
                nc.vector.tensor_tensor(out=gch[:], in0=gch[:], in1=gch[:],
                                        op=ALU.mult)

                # ---- R^2 via PE, then sqrt
                rsp = pool.tile([8, CI], F32, tag="rsp", bufs=1)
                if "mm" in ablate:
                    nc.vector.memset(rsp[0:8, :], 1.0)
                for h in range(4 if "mm" not in ablate else 0):
                    ps = psum.tile([8, 1024], F32, tag="pchunk", bufs=2)
                    for j in range(2):
                        c0 = 1024 * h + 512 * j
                        nc.tensor.matmul(out=ps[:, 512 * j:512 * j + 512],
                                         lhsT=wq[:], rhs=gch[:, c0:c0 + 512],
                                         start=True, stop=True)
                    nc.scalar.activation(out=rsp[0:8, 1024 * h:1024 * h + 1024],
                                         in_=ps[:], func=AF.Sqrt)
                # round-trip compaction [8, 4096] -> [128, 256]
                nc.sync.dma_start(out=rr_dram[k], in_=rsp[0:8, :])
                nc.sync.dma_start(
                    out=Rt[:, 256 * k:256 * (k + 1)],
                    in_=rr_dram[k].rearrange("g (p f) -> (g p) f", p=16))

                stage_map = {1: (0, 8), 3: (512, 8), 5: (1024, 8),
                             6: (1536, 4), 7: (1792, 4)}
                if k in stage_map and "quarter" not in ablate:
                    c0, nsc = stage_map[k]
                    _quarter(nc, pool, psum, Rt, zc, sym, bnred, bnbc, cb,
                             out_ext, c0, nsc, rc_list, rcg_of_l, rs_v, re_v)


def _quarter(nc, pool, psum, Rt, zc, sym, bnred, bnbc, cb, out_ext,
             c0, nsc, rc_list, rcg_of_l, rs_v, re_v):
    """rsf + masked reduce + BN for R columns [c0, c0 + 64*nsc).

    sym is the transposed accumulator [(stage, l, a, ns) blocks]; the final
    BN multiply writes through a strided AP to restore (ns, a, l) order.
    """
    W = 64 * nsc
    fsl = slice(c0, c0 + W)
    ns0 = c0 // 64
    rsq = pool.tile([P, 512], F32, tag="rsq", bufs=1)
    nc.vector.tensor_tensor(out=rsq[:, 0:W], in0=Rt[:, fsl], in1=Rt[:, fsl],
                            op=ALU.mult)

    c1s = []
    for gi, rcval in enumerate(rc_list):
        ur = pool.tile([P, 512], F32, tag="ur", bufs=1)
        nc.scalar.activation(out=ur[:, 0:W], in_=Rt[:, fsl], func=AF.Relu,
                             scale=-PI / rcval, bias=cb[:, 0:1])
        c1 = pool.tile([P, 512], F32, tag=f"c1_{gi}")
        nc.scalar.activation(out=c1[:, 0:W], in_=ur[:, 0:W], func=AF.Sin,
                             scale=-1.0, bias=cb[:, 1:2])  # sin(pi/2-u)=cos(u)
        c1s.append(c1)

    # 4 type masks packed [128, (a, i)] so each l needs ONE mask multiply
    mask4 = pool.tile([P, 4 * 512], F32, tag="mask4", bufs=1)
    for a in range(A):
        nc.vector.tensor_scalar(out=mask4[:, 512 * a:512 * a + W],
                                in0=zc[:, fsl],
                                scalar1=float(ATOM_TYPES[a]), scalar2=None,
                                op0=ALU.is_equal)

    for l in range(L):
        u = pool.tile([P, 512], F32, tag="u", bufs=2)
        nc.vector.scalar_tensor_tensor(
            out=u[:, 0:W], in0=Rt[:, fsl], scalar=-2.0 * rs_v[l],
            in1=rsq[:, 0:W], op0=ALU.mult, op1=ALU.add)
        kp = pool.tile([P, 512], F32, tag="kp", bufs=2)
        nc.scalar.activation(out=kp[:, 0:W], in_=u[:, 0:W], func=AF.Exp,
                             scale=-re_v[l], bias=cb[:, 3 + l:4 + l])
        rsf = pool.tile([P, 512], F32, tag="rsf", bufs=2)
        nc.vector.scalar_tensor_tensor(
            out=rsf[:, 0:W], in0=c1s[rcg_of_l[l]][:, 0:W], scalar=1.0,
            in1=kp[:, 0:W], op0=ALU.subtract, op1=ALU.mult)  # -K'*FCx2
        # one multiply for all 4 type masks: rsf broadcast over the a axis
        pm4 = pool.tile([P, 4 * 512], F32, tag="pm4", bufs=1)
        rsf_b = bass.AP(rsf.tensor, rsf[:].offset,
                        [[rsf[:].ap[0][0], P], [0, 4], [1, W]])
        pm4_w = bass.AP(pm4.tensor, pm4[:].offset,
                        [[pm4[:].ap[0][0], P], [512, 4], [1, W]])
        nc.vector.tensor_tensor(out=pm4_w, in0=rsf_b, in1=bass.AP(
            mask4.tensor, mask4[:].offset,
            [[mask4[:].ap[0][0], P], [512, 4], [1, W]]), op=ALU.mult)
        # one segmented reduce -> contiguous [128, (a, ns)] block of sym
        base = 48 * ns0 + l * 4 * nsc
        pm4_r = bass.AP(pm4.tensor, pm4[:].offset,
                        [[pm4[:].ap[0][0], P], [512, 4], [64, nsc], [1, 64]])
        nc.vector.tensor_reduce(
            out=sym[:, base:base + 4 * nsc], in_=pm4_r,
            axis=mybir.AxisListType.X, op=ALU.add)

    # ---- batch-norm for this stage's 48*nsc sym cols [(l, a, ns) layout]
    CW = 48 * nsc
    cf = slice(48 * ns0, 48 * ns0 + CW)
    ssq = pool.tile([P, 384], F32, tag="ssq", bufs=1)
    nc.vector.tensor_tensor(out=ssq[:, 0:CW], in0=sym[:, cf], in1=sym[:, cf],
                            op=ALU.mult)
    pm1 = psum.tile([8, 384], F32, tag="pbn0")
    nc.tensor.matmul(out=pm1[:, 0:CW], lhsT=bnred[:], rhs=sym[:, cf],
                     start=True, stop=True)
    pm2 = psum.tile([8, 384], F32, tag="pbn1")
    nc.tensor.matmul(out=pm2[:, 0:CW], lhsT=bnred[:], rhs=ssq[:, 0:CW],
                     start=True, stop=True)
    msb = pool.tile([8, 384], F32, tag="msb", bufs=1)
    nc.vector.tensor_copy(out=msb[0:8, 0:CW], in_=pm1[:, 0:CW])
    m2 = pool.tile([8, 384], F32, tag="m2", bufs=1)
    nc.vector.tensor_tensor(out=m2[0:8, 0:CW], in0=msb[0:8, 0:CW],
                            in1=msb[0:8, 0:CW], op=ALU.mult)
    vsb = pool.tile([8, 384], F32, tag="vsb", bufs=1)
    nc.vector.tensor_tensor(out=vsb[0:8, 0:CW], in0=pm2[:, 0:CW],
                            in1=m2[0:8, 0:CW], op=ALU.subtract)
    ssb = pool.tile([8, 384], F32, tag="ssb", bufs=1)
    nc.scalar.activation(out=ssb[0:8, 0:CW], in_=vsb[0:8, 0:CW], func=AF.Sqrt,
                         bias=cb[0:8, 2:3])
    rsb = pool.tile([8, 384], F32, tag="rsb", bufs=1)
    nc.vector.reciprocal(out=rsb[0:8, 0:CW], in_=ssb[0:8, 0:CW])
    pbm = psum.tile([P, 384], F32, tag="pbn2")
    nc.tensor.matmul(out=pbm[:, 0:CW], lhsT=bnbc[:], rhs=msb[0:8, 0:CW],
                     start=True, stop=True)
    pbr = psum.tile([P, 384], F32, tag="pbn3")
    nc.tensor.matmul(out=pbr[:, 0:CW], lhsT=bnbc[:], rhs=rsb[0:8, 0:CW],
                     start=True, stop=True)
    dsb = pool.tile([P, 384], F32, tag="dsb", bufs=1)
    nc.vector.tensor_tensor(out=dsb[:, 0:CW], in0=pbm[:, 0:CW], in1=sym[:, cf],
                            op=ALU.subtract)
    # final multiply writes transposed: (l, a, ns) walk -> col ns*48 + a*12 + l
    osb = pool.tile([P, 384], F32, tag="osb", bufs=2)
    dsb_v = dsb[:, 0:CW].rearrange("p (l a s) -> p l a s", l=12, a=4)
    pbr_v = pbr[:, 0:CW].rearrange("p (l a s) -> p l a s", l=12, a=4)
    osb_w = bass.AP(osb.tensor, osb[:].offset,
                    [[osb[:].ap[0][0], P], [1, 12], [12, 4], [48, nsc]])
    nc.vector.tensor_tensor(out=osb_w, in0=dsb_v, in1=pbr_v, op=ALU.mult)
    nc.sync.dma_start(out=out_ext[:, cf], in_=osb[:, 0:CW])


# ---------------------------------------------------------------- host side

def make_cbias(rs_v, re_v):
    cb = np.zeros((P, 16), np.float32)
    cb[:, 0] = PI
    cb[:, 1] = 0.5 * PI
    cb[:, 2] = BN_EPS
    cb[:, 15] = 1e-4
    for l in range(L):
        cb[:, 3 + l] = -float(re_v[l]) * float(rs_v[l]) ** 2 + math.log(0.5)
    return cb


def prep_core_inputs(X, Nbrs, Nbrs_Z, r, const_cache={}):
    """Build core r's input map (numpy layout prep only)."""
    n0 = NSH * r
    Xt = np.ascontiguousarray(X.transpose(2, 0, 1))          # [3, B, N]
    if "tbl" not in const_cache:
        tbl = np.zeros((8, 16, 2, N), np.float32)
        tbl[:, 0:3, :, :] = Xt.reshape(3, 8, 2, N).transpose(1, 0, 2, 3)
        const_cache["tbl"] = tbl.reshape(P, 2 * N)

        wq = np.zeros((P, 8), np.float32)
        for g in range(8):
            wq[16 * g + 0:16 * g + 3, g] = 1.0
        bnred = np.zeros((P, 8), np.float32)
        bnbc = np.zeros((8, P), np.float32)
        for p in range(P):
            bnred[p, p % 8] = 1.0 / 16.0
            bnbc[p % 8, p] = 1.0
        const_cache["wq"] = wq
        const_cache["bnred"] = bnred
        const_cache["bnbc"] = bnbc
        const_cache["cbias"] = None  # filled by caller

    cen = np.zeros((8, 16, 2, NSH), np.float32)
    cen[:, 0:3, :, :] = (Xt[:, :, n0:n0 + NSH]
                         .reshape(3, 8, 2, NSH).transpose(1, 0, 2, 3))
    cen = cen.reshape(P, 2 * NSH)

    nbr_sh = Nbrs[:, n0:n0 + NSH, :]                          # [16, 256, 64]
    nbr6 = nbr_sh.reshape(8, 2, 8, 8, 4, M)                   # [g, beta, nb, k, j, m]
    lg = nbr6 + (np.arange(2, dtype=nbr6.dtype)
                 .reshape(1, 2, 1, 1, 1, 1) * N)
    lg = lg.transpose(0, 3, 1, 2, 4, 5).reshape(8, NCHUNK * CI)
    gidx = (lg.reshape(8, TF, 16).transpose(0, 2, 1)
            .reshape(P, TF).astype(np.int16))

    zc = (Nbrs_Z[:, n0:n0 + NSH, :].reshape(8, 2, 8, 32, M)
          .reshape(P, TF).astype(np.float32))

    return {"tbl": const_cache["tbl"], "gidx": gidx, "cen": cen, "zc": zc,
            "wq": const_cache["wq"], "bnred": const_cache["bnred"],
            "bnbc": const_cache["bnbc"], "cbias": const_cache["cbias"]}


def assemble_output(results):
    full = np.empty((8, 2, N, NFEAT), np.float32)             # [g, beta, n, f]
    for r in range(8):
        o = np.asarray(results[r]["out"]).reshape(8, 2, NSH, NFEAT)
        n0 = NSH * r
        full[:, :, n0:n0 + NSH, :] = o
    return full.reshape(B, N, NFEAT)


_cache = {}


def kernel(X, Nbrs, Nbrs_Z, rc, rs, re):
    from concourse.bass_utils import run_bass_kernel_spmd
    key = (tuple(np.asarray(rc).ravel().tolist()),
           tuple(np.asarray(rs).ravel().tolist()),
           tuple(np.asarray(re).ravel().tolist()))
    if key not in _cache:
        _cache[key] = build_nc(np.asarray(rc).ravel(), np.asarray(rs).ravel(),
                               np.asarray(re).ravel())
    nc = _cache[key]
    X = np.asarray(X, np.float32)
    Nbrs = np.asarray(Nbrs)
    Nbrs_Z = np.asarray(Nbrs_Z)
    cc = {}
    in_maps = [prep_core_inputs(X, Nbrs, Nbrs_Z, r, cc) for r in range(8)]
    cbias = make_cbias(np.asarray(rs).ravel(), np.asarray(re).ravel())
    for im in in_maps:
        im["cbias"] = cbias
    res = run_bass_kernel_spmd(nc, in_maps, core_ids=list(range(8)))
    return assemble_output(res.results)
